# revision 9
# baseline (speedup 1.0000x reference)
"""Bipartite GNN attention kernel for Trainium2, SPMD across 8 NeuronCores.

Math (per reference):
  u = user @ W_u.T + b_u ; v = item @ W_v.T + b_v
  learn_user = softmax((u @ v.T) * UV_adj * scale, axis=1) @ v + u
  learn_item = softmax((v @ u.T) * VU_adj * scale, axis=1) @ u + v

Sharding: core i owns rows [i*1024, (i+1)*1024) of BOTH outputs; no
collectives (the contracted-side projection is replicated).

v3 design (fp8 DoubleRow):
- All big matmuls (scores, aggregation, denominator, projections) run in
  fp8e4 with perf_mode=DoubleRow (2 k-chunks per instruction, ~1.5x PE).
- Feature matrices are projected twice: fT [h, N] (feature-major, biased,
  used as score lhsT) and vrow [N, h] (row-major, UNbiased, used as
  aggregation rhs). The missing bias in vrow cancels through softmax:
  P@(v + 1 b^T)/rsum = P@vrow/rsum + b^T, so b_feat is folded into the
  residual qrow instead. This removes all per-block PE transposes.
- Per-core inputs are column-ROLLED so this core's rows are columns
  [0:RB) of both feature matrices; the score rhs (qTb) is then just
  fT_other[:, :, 0:RB] - no separate query projection.
- exp uses bias -ln(32): softmax is shift-invariant, masked entries
  become exactly 1/32 (fp8-exact), max value ~5 stays far below fp8e4
  max 240.
- Residual path stays accurate: qrow = f32r projection of the f32 query
  rows -> bf16, + (b_q + b_feat) broadcast row.
- Aggregation of pair bp-1 is emitted after scores of pair bp so the PE
  never waits on the DVE-mult + Act-exp chain.
"""

import sys

sys.path.insert(0, "/opt/trn_rl_repo")

import ml_dtypes
import numpy as np

import concourse.bacc as bacc
import concourse.bass as bass
import concourse.mybir as mybir
import concourse.tile as tile
from concourse.bass_utils import run_bass_kernel_spmd

N = 8192          # users == items
H = 512           # hidden
NCORES = 8
RB = N // NCORES  # 1024 rows per core per direction
KH = H // 128     # 4 h-chunks
NB = N // 128     # 64 column chunks
NBP = NB // 2     # 32 column-pair chunks (DoubleRow)
RBQ = 256         # users per attention row-block
NRB = RB // RBQ   # 4 r-blocks of 256
NJ = N // 512     # 16 512-col blocks for projection streaming
SCALE = float(1.0 / np.sqrt(np.float32(H)))
NLN32 = float(-np.log(32.0))

F32 = mybir.dt.float32
F32R = mybir.dt.float32r
BF16 = mybir.dt.bfloat16
FP8 = mybir.dt.float8e4
NP_FP8 = ml_dtypes.float8_e4m3
DR = mybir.MatmulPerfMode.DoubleRow


def _r(ap):
    return ap.bitcast(F32R)


def build_nc():
    nc = bacc.Bacc("TRN2", target_bir_lowering=False, debug=False)

    featA = nc.declare_dram_parameter("featA", [H, N], FP8, isOutput=False)
    featB = nc.declare_dram_parameter("featB", [H, N], FP8, isOutput=False)
    qtA = nc.declare_dram_parameter("qtA", [H, RB], F32, isOutput=False)
    qtB = nc.declare_dram_parameter("qtB", [H, RB], F32, isOutput=False)
    maskA = nc.declare_dram_parameter("maskA", [N, RB], FP8, isOutput=False)
    maskB = nc.declare_dram_parameter("maskB", [N, RB], FP8, isOutput=False)
    WfA = nc.declare_dram_parameter("WfA", [128, KH, H], FP8, isOutput=False)
    WfB = nc.declare_dram_parameter("WfB", [128, KH, H], FP8, isOutput=False)
    WqA = nc.declare_dram_parameter("WqA", [H, H], F32, isOutput=False)
    WqB = nc.declare_dram_parameter("WqB", [H, H], F32, isOutput=False)
    bfA = nc.declare_dram_parameter("bfA", [128, KH], F32, isOutput=False)
    bfB = nc.declare_dram_parameter("bfB", [128, KH], F32, isOutput=False)
    brow = nc.declare_dram_parameter("brow", [128, H], F32, isOutput=False)
    out = nc.declare_dram_parameter("out", [2 * RB, H], F32, isOutput=True)

    with tile.TileContext(nc) as tc:
        with (
            tc.tile_pool(name="bigA", bufs=1) as bigA,
            tc.tile_pool(name="bigB", bufs=1) as bigB,
            tc.tile_pool(name="wts", bufs=1) as wts,
            tc.tile_pool(name="stream", bufs=6) as stream,
            tc.tile_pool(name="qstream", bufs=4) as qstream,
            tc.tile_pool(name="mask", bufs=3) as maskp,
            tc.tile_pool(name="pf", bufs=3) as pfp,
            tc.tile_pool(name="pb", bufs=3) as pbp,
            tc.tile_pool(name="outs", bufs=1) as outsp,
            tc.tile_pool(name="small", bufs=1) as small,
            tc.tile_pool(name="ps_s", bufs=3, space="PSUM") as ps_s,      # 3 banks
            tc.tile_pool(name="ps_agg", bufs=1, space="PSUM") as ps_agg,  # 2 banks
            tc.tile_pool(name="ps_rs", bufs=1, space="PSUM") as ps_rs,    # 1 bank
            tc.tile_pool(name="ps_aux", bufs=1, space="PSUM") as ps_aux,  # 1 bank
        ):
            ones2 = small.tile([128, 2, 16], FP8, tag="ones")
            nc.vector.memset(ones2[:], 1.0)
            onesf = small.tile([1, 1], F32, tag="onesf")
            nc.vector.memset(onesf[:], 1.0)
            nbias = small.tile([128, 1], F32, tag="nbias")
            nc.vector.memset(nbias[:], NLN32)
            brow_sb = small.tile([128, H], F32, tag="brow")
            nc.sync.dma_start(brow_sb[:], brow[:])
            bfA_sb = small.tile([128, KH], F32, tag="bfA")
            nc.sync.dma_start(bfA_sb[:], bfA[:])
            bfB_sb = small.tile([128, KH], F32, tag="bfB")
            nc.sync.dma_start(bfB_sb[:], bfB[:])

            # persistent per-direction tensors
            fT = {}
            vrow = {}
            qrow = {}
            for big_pool, d in ((bigA, "A"), (bigB, "B")):
                fT[d] = big_pool.tile([128, KH, N], FP8, tag=f"fT{d}",
                                      name=f"fT{d}")
                vrow[d] = big_pool.tile([128, NB, H], FP8, tag=f"vrow{d}",
                                        name=f"vrow{d}")
                qrow[d] = big_pool.tile([128, 2 * KH, H], BF16, tag=f"qrow{d}",
                                        name=f"qrow{d}")

            # ---------------- phase 0: projections ----------------
            def project(d, feat_dram, qt_dram, wf_dram, wq_dram, bias_f):
                wfp = wts.tile([128, KH, H], FP8, tag="wfp", name=f"wfp{d}")
                nc.sync.dma_start(wfp[:], wf_dram[:])
                wq = [wts.tile([128, H], F32R, tag=f"wq{k}", name=f"wq{d}{k}")
                      for k in range(KH)]
                for k in range(KH):
                    nc.sync.dma_start(
                        wq[k][:], wq_dram[k * 128:(k + 1) * 128, :].bitcast(F32R))

                # qrow: residual projection, f32r for accuracy
                qt_in = [qstream.tile([128, RB], F32R, tag="qt",
                                      name=f"qt{d}{k}") for k in range(KH)]
                for k in range(KH):
                    nc.sync.dma_start(
                        qt_in[k][:],
                        qt_dram[k * 128:(k + 1) * 128, :].bitcast(F32R))
                for c in range(2 * KH):
                    ps = ps_s.tile([128, H], F32, tag="s")
                    for k in range(KH):
                        nc.tensor.matmul(
                            ps[:], qt_in[k][:, c * 128:(c + 1) * 128], wq[k][:],
                            start=(k == 0), stop=(k == KH - 1))
                    nc.vector.tensor_tensor(
                        out=qrow[d][:, c, :], in0=ps[:], in1=brow_sb[:],
                        op=mybir.AluOpType.add)

                # fT + vrow: fp8 DoubleRow projections
                for j in range(NJ):
                    ft_in = stream.tile([128, KH, 512], FP8, tag="ft",
                                        name=f"ft{d}{j}")
                    for k in range(KH):
                        nc.sync.dma_start(
                            ft_in[:, k, :],
                            feat_dram[k * 128:(k + 1) * 128,
                                      j * 512:(j + 1) * 512])
                    for m in range(KH):
                        ps = ps_s.tile([128, 512], F32, tag="s")
                        for ko in range(2):
                            nc.tensor.matmul(
                                ps[:],
                                wfp[:, 2 * ko:2 * ko + 2, m * 128:(m + 1) * 128],
                                ft_in[:, 2 * ko:2 * ko + 2, :],
                                start=(ko == 0), stop=(ko == 1), perf_mode=DR)
                        if m % 2 == 0:
                            nc.vector.tensor_scalar(
                                out=fT[d][:, m, j * 512:(j + 1) * 512],
                                in0=ps[:], scalar1=bias_f[:, m:m + 1],
                                scalar2=None, op0=mybir.AluOpType.add)
                        else:
                            nc.scalar.add(
                                fT[d][:, m, j * 512:(j + 1) * 512], ps[:],
                                bias_f[:, m:m + 1])
                    for sub in range(4):
                        c = j * 4 + sub
                        ps = ps_s.tile([128, 512], F32, tag="s")
                        for ko in range(2):
                            nc.tensor.matmul(
                                ps[:],
                                ft_in[:, 2 * ko:2 * ko + 2,
                                      sub * 128:(sub + 1) * 128],
                                wfp[:, 2 * ko:2 * ko + 2, :],
                                start=(ko == 0), stop=(ko == 1), perf_mode=DR)
                        if sub % 2 == 0:
                            nc.vector.tensor_copy(vrow[d][:, c, :], ps[:])
                        else:
                            nc.scalar.copy(vrow[d][:, c, :], ps[:])

            project("A", featA, qtA, WfA, WqA, bfA_sb)
            project("B", featB, qtB, WfB, WqB, bfB_sb)

            # ---------------- attention ----------------
            def emit_agg(myvrow, agg, rs_row, pbf2, bp):
                for rs in range(2):
                    nc.tensor.matmul(
                        agg[:, rs, :], pbf2[:, :, rs * 128:(rs + 1) * 128],
                        myvrow[:, 2 * bp:2 * bp + 2, :],
                        start=(bp == 0), stop=(bp == NBP - 1), perf_mode=DR)
                # denominator: ones^T @ pbf2 -> [1, RBQ] row; ones is the
                # stationary operand so the weight load is only 2 columns
                nc.tensor.matmul(
                    rs_row[:], ones2[:, :, 0:1], pbf2[:],
                    start=(bp == 0), stop=(bp == NBP - 1), perf_mode=DR)

            def attention(d, other, mask_dram, out_base):
                myfT = fT[d]
                myvrow = vrow[d]
                qTb = fT[other]
                for rb in range(NRB):
                    agg = ps_agg.tile([128, 2, 512], F32, tag="agg")
                    rs_row = ps_rs.tile([1, RBQ], F32, tag="rs")
                    pend = None  # pbf2 pair waiting for aggregation
                    for bp in range(NBP):
                        # both score halves share one psum bank = ONE
                        # accumulation group: start on first, stop on last
                        sps = ps_s.tile([128, 2, RBQ], F32, tag="s")
                        for t in range(2):
                            b = 2 * bp + t
                            for ko in range(2):
                                nc.tensor.matmul(
                                    sps[:, t, :],
                                    myfT[:, 2 * ko:2 * ko + 2,
                                         b * 128:(b + 1) * 128],
                                    qTb[:, 2 * ko:2 * ko + 2,
                                        rb * RBQ:(rb + 1) * RBQ],
                                    start=(t == 0 and ko == 0),
                                    stop=(t == 1 and ko == 1),
                                    perf_mode=DR)

                        # aggregate previous pair while DVE/Act chew on this one
                        if pend is not None:
                            emit_agg(myvrow, agg, rs_row, *pend)
                        mt = maskp.tile([128, 2, RBQ], FP8, tag="mk")
                        for t in range(2):
                            nc.sync.dma_start(
                                mt[:, t, :],
                                mask_dram[(2 * bp + t) * 128:
                                          (2 * bp + t + 1) * 128,
                                          rb * RBQ:(rb + 1) * RBQ])
                        pbf2 = pbp.tile([128, 2, RBQ], FP8, tag="pbf")
                        p16 = pfp.tile([128, 2, RBQ], BF16, tag="p16")
                        nc.vector.tensor_tensor(
                            out=p16[:], in0=sps[:], in1=mt[:],
                            op=mybir.AluOpType.mult)
                        nc.scalar.activation(
                            pbf2[:], p16[:],
                            mybir.ActivationFunctionType.Exp,
                            bias=nbias[:], scale=SCALE)
                        pend = (pbf2, bp)
                    emit_agg(myvrow, agg, rs_row, *pend)

                    # epilogue: transpose [1, RBQ] denominators to [128, 2]
                    # via two 1-partition matmuls, then out = agg/rsum + qrow
                    rs_sb = small.tile([1, RBQ], F32, tag="rs_sb")
                    nc.vector.tensor_copy(rs_sb[:], rs_row[:])
                    rsT = ps_aux.tile([128, 2], F32, tag="rsT")
                    for rs in range(2):
                        nc.tensor.matmul(
                            rsT[:, rs:rs + 1],
                            rs_sb[0:1, rs * 128:(rs + 1) * 128], onesf[:],
                            start=(rs == 0), stop=(rs == 1))
                    recip = small.tile([128, 2], F32, tag="recip")
                    nc.vector.reciprocal(recip[:], rsT[:])
                    for rs in range(2):
                        o_sb = outsp.tile([128, H], F32, tag=f"o{rs}",
                                          name=f"o{d}{rb}_{rs}")
                        if rs % 2 == 0:
                            nc.vector.tensor_scalar(
                                out=o_sb[:], in0=agg[:, rs, :],
                                scalar1=recip[:, rs:rs + 1], scalar2=None,
                                op0=mybir.AluOpType.mult)
                            nc.vector.tensor_tensor(
                                out=o_sb[:], in0=o_sb[:],
                                in1=qrow[d][:, rb * 2 + rs, :],
                                op=mybir.AluOpType.add)
                        else:
                            nc.scalar.mul(o_sb[:], agg[:, rs, :],
                                          recip[:, rs:rs + 1])
                            nc.gpsimd.tensor_tensor(
                                out=o_sb[:], in0=o_sb[:],
                                in1=qrow[d][:, rb * 2 + rs, :],
                                op=mybir.AluOpType.add)
                        row0 = out_base + rb * RBQ + rs * 128
                        nc.sync.dma_start(out[row0:row0 + 128, :], o_sb[:])

            attention("A", "B", maskA, 0)
            attention("B", "A", maskB, RB)

    nc.compile()
    return nc


_NC_CACHE = None
TRACE = False
LAST_RESULT = None


def kernel(user, item, UV_adj, VU_adj, W_u, b_u, W_v, b_v):
    global _NC_CACHE, LAST_RESULT
    user = np.asarray(user, dtype=np.float32)
    item = np.asarray(item, dtype=np.float32)
    UV_adj = np.asarray(UV_adj, dtype=np.float32)
    VU_adj = np.asarray(VU_adj, dtype=np.float32)
    W_u = np.asarray(W_u, dtype=np.float32)
    W_v = np.asarray(W_v, dtype=np.float32)
    b_u = np.asarray(b_u, dtype=np.float32)
    b_v = np.asarray(b_v, dtype=np.float32)

    userT = np.ascontiguousarray(user.T)
    itemT = np.ascontiguousarray(item.T)
    userT8 = userT.astype(NP_FP8)
    itemT8 = itemT.astype(NP_FP8)
    UV8 = UV_adj.astype(NP_FP8)
    VU8 = np.ascontiguousarray(UV8.T)
    W_uT = np.ascontiguousarray(W_u.T)
    W_vT = np.ascontiguousarray(W_v.T)
    # [128, KH, H] fp8 weight layout for DoubleRow projections
    WfA_np = np.ascontiguousarray(
        W_vT.reshape(KH, 128, H).transpose(1, 0, 2).astype(NP_FP8))
    WfB_np = np.ascontiguousarray(
        W_uT.reshape(KH, 128, H).transpose(1, 0, 2).astype(NP_FP8))
    bfA_np = np.ascontiguousarray(b_v.reshape(KH, 128).T)
    bfB_np = np.ascontiguousarray(b_u.reshape(KH, 128).T)
    brow_np = np.ascontiguousarray(
        np.broadcast_to((b_u + b_v)[None, :], (128, H)))

    in_maps = []
    for i in range(NCORES):
        r = i * RB
        sl = slice(r, r + RB)
        in_maps.append({
            # feature matrices with this core's rows rolled to the front
            "featA": np.ascontiguousarray(np.roll(itemT8, -r, axis=1)),
            "featB": np.ascontiguousarray(np.roll(userT8, -r, axis=1)),
            "qtA": np.ascontiguousarray(userT[:, sl]),
            "qtB": np.ascontiguousarray(itemT[:, sl]),
            "maskA": np.ascontiguousarray(np.roll(VU8[:, sl], -r, axis=0)),
            "maskB": np.ascontiguousarray(np.roll(UV8[:, sl], -r, axis=0)),
            "WfA": WfA_np,
            "WfB": WfB_np,
            "WqA": W_uT,
            "WqB": W_vT,
            "bfA": bfA_np,
            "bfB": bfB_np,
            "brow": brow_np,
        })

    if _NC_CACHE is None:
        _NC_CACHE = build_nc()
    res = run_bass_kernel_spmd(_NC_CACHE, in_maps, core_ids=list(range(NCORES)),
                               trace=TRACE)
    LAST_RESULT = res
    results = res.results
    learn_user = np.concatenate([results[i]["out"][:RB] for i in range(NCORES)], 0)
    learn_item = np.concatenate([results[i]["out"][RB:] for i in range(NCORES)], 0)
    return (learn_user, learn_item)


if __name__ == "__main__":
    nc = build_nc()
    print("built ok")
